# revision 1
# baseline (speedup 1.0000x reference)
"""Conv2d(128->256, 3x3, pad 1, stride 1) on 32x56x56 fp32, for 8 trn2 cores.

Strategy: data-parallel over batch N=32 -> 4 images/core. Per core an
implicit-GEMM conv: C_in=128 is the partition (contraction) dim; for each
(kh, kw) tap a [128ci x 128co] weight tile multiplies a shifted window of the
zero-padded input image held in SBUF, accumulating into PSUM over the 9 taps.
Output rows are processed in chunks of 8 (free dim 8*56=448 <= 512 PSUM bank).
Matmuls run in float16 (inputs ~N(0,0.03..1): fp16 keeps ~2.6e-4 rel err) with fp32 PSUM accumulate; fp16 enables fast weight load so the 504 LDWEIGHTS hide under the matmul stream.

Rings: SP carries x input, ACT carries weights/bias + half-0 outputs,
SWDGE(Pool) carries half-1 outputs. Weights are laid out half-major on the
host so the half-0 weight DMA (the first-matmul gate) is small and lands
first; image-0's top rows are split into two DMAs for the same reason.
Staging pools use bufs=1 so later images' loads queue behind the cast that
frees the slot instead of competing with the critical head transfers.
"""
import numpy as np
from contextlib import ExitStack

N_FULL, C_IN, H, W = 32, 128, 56, 56
C_OUT, KS = 256, 3
N_CORES = 8
N_PER = N_FULL // N_CORES          # 4 images per core
HP = H + 2                          # 58 padded
PIX = H * W                         # 3136
ROWS = 8                            # output rows per psum chunk
RC = H // ROWS                      # 7 chunks
NF = ROWS * W                       # 448 free elems per matmul

T_ROWS = 34                         # xpad_top: padded rows 0..33  (chunks 0-3)
B_ROWS = 26                         # xpad_bot: padded rows 32..57 (chunks 4-6)
XT_R = 33                           # x rows 0..32 feed top interior
XT_A = 17                           # first sub-DMA: x rows 0..16
XT_B = XT_R - XT_A                  # second sub-DMA: x rows 17..32
XB_R = 25                           # x rows 31..55 feed bottom interior

_CACHE = {}


def _build():
    import concourse.tile as tile
    from concourse import mybir, bacc

    f32 = mybir.dt.float32
    f16 = mybir.dt.float16

    nc = bacc.Bacc("TRN2", target_bir_lowering=False, debug=False)
    x_d = nc.dram_tensor("x", [N_PER, C_IN, H, W], f16, kind="ExternalInput").ap()
    # host-pretransposed: [ci, half, k, co_half] (half-major, contiguous per half)
    w_d = nc.dram_tensor("w", [C_IN, 2, KS * KS, 128], f16, kind="ExternalInput").ap()
    b_d = nc.dram_tensor("b", [C_OUT], f32, kind="ExternalInput").ap()
    y_d = nc.dram_tensor("y", [N_PER, C_OUT, H, W], f32, kind="ExternalOutput").ap()

    with tile.TileContext(nc) as tc:
        with ExitStack() as ctx:
            wp = ctx.enter_context(tc.tile_pool(name="wp", bufs=1))
            xrawta = ctx.enter_context(tc.tile_pool(name="xrawta", bufs=1))
            xrawtb = ctx.enter_context(tc.tile_pool(name="xrawtb", bufs=1))
            xrawb = ctx.enter_context(tc.tile_pool(name="xrawb", bufs=1))
            xpadt = ctx.enter_context(tc.tile_pool(name="xpadt", bufs=2))
            xpadb = ctx.enter_context(tc.tile_pool(name="xpadb", bufs=2))
            pp = ctx.enter_context(tc.tile_pool(name="pp", bufs=4, space="PSUM"))
            op = ctx.enter_context(tc.tile_pool(name="op", bufs=2))

            # Weight half 0 first on the ACT ring: it gates the first matmul.
            # Half 1 and bias are issued after image-0's input DMAs so they
            # don't sit ahead of them in the ring FIFOs.
            w_r = wp.tile([C_IN, 2 * KS * KS * 128], f16)
            w_r4 = w_r[:].rearrange("p (h k co) -> p h k co", h=2, k=KS * KS)
            nc.scalar.dma_start(
                w_r4[:, 0], w_d[:, 0].rearrange("ci k co -> ci (k co)")
            )

            # PE warmup: ~3.4us of dummy matmuls while the head DMAs land, so
            # the HAM clock gate opens before the first real matmul issues.
            wu = wp.tile([128, 448], f16)
            nc.vector.memset(wu[:], 0.0)
            wups = pp.tile([128, NF], f32, tag="ps")
            for _ in range(9):
                nc.tensor.matmul(wups[:], wu[:, 0:128], wu[:], start=True, stop=True)

            bias_sb = wp.tile([128, 2], f32)

            for n in range(N_PER):
                # top interior in two slices so the first chunks unblock early
                xrta = xrawta.tile([C_IN, XT_A * W], f16)
                nc.sync.dma_start(xrta[:], x_d[n, :, 0:XT_A, :].rearrange("c h w -> c (h w)"))
                xrtb = xrawtb.tile([C_IN, XT_B * W], f16)
                nc.sync.dma_start(xrtb[:], x_d[n, :, XT_A:XT_R, :].rearrange("c h w -> c (h w)"))
                # bottom: x rows 31..55 -> padded rows 32..56 (local 0..24)
                xrb = xrawb.tile([C_IN, XB_R * W], f16)
                nc.sync.dma_start(xrb[:], x_d[n, :, 31 : 31 + XB_R, :].rearrange("c h w -> c (h w)"))

                if n == 0:
                    # now that image-0's loads are queued: weight half 1 + bias
                    nc.scalar.dma_start(
                        w_r4[:, 1], w_d[:, 1].rearrange("ci k co -> ci (k co)")
                    )
                    nc.scalar.dma_start(bias_sb[:], b_d.rearrange("(h p) -> p h", h=2))

                xpt = xpadt.tile([C_IN, T_ROWS * HP], f16)
                xpt3 = xpt[:].rearrange("p (a b) -> p a b", a=T_ROWS)
                nc.vector.memset(xpt3[:, 0, :], 0.0)
                nc.vector.memset(xpt3[:, 1:T_ROWS, 0:1], 0.0)
                nc.vector.memset(xpt3[:, 1:T_ROWS, HP - 1 : HP], 0.0)
                nc.vector.tensor_copy(
                    xpt3[:, 1 : 1 + XT_A, 1 : 1 + W],
                    xrta[:].rearrange("p (a b) -> p a b", a=XT_A),
                )
                nc.vector.tensor_copy(
                    xpt3[:, 1 + XT_A : 1 + XT_R, 1 : 1 + W],
                    xrtb[:].rearrange("p (a b) -> p a b", a=XT_B),
                )

                xpb = xpadb.tile([C_IN, B_ROWS * HP], f16)
                xpb3 = xpb[:].rearrange("p (a b) -> p a b", a=B_ROWS)
                nc.vector.memset(xpb3[:, B_ROWS - 1, :], 0.0)
                nc.vector.memset(xpb3[:, 0 : B_ROWS - 1, 0:1], 0.0)
                nc.vector.memset(xpb3[:, 0 : B_ROWS - 1, HP - 1 : HP], 0.0)
                nc.vector.tensor_copy(
                    xpb3[:, 0 : B_ROWS - 1, 1 : 1 + W],
                    xrb[:].rearrange("p (a b) -> p a b", a=XB_R),
                )

                out_sb = op.tile([128, 2 * PIX], f32)
                last_img = n == N_PER - 1
                for half in range(2):
                    for rc in range(RC):
                        ps = pp.tile([128, NF], f32)
                        for kh in range(KS):
                            for kw in range(KS):
                                k = kh * KS + kw
                                lhsT = w_r4[:, half, k, :]
                                if rc < 4:
                                    rhs = xpt3[:, rc * ROWS + kh : rc * ROWS + kh + ROWS, kw : kw + W]
                                else:
                                    lr = (rc - 4) * ROWS + kh
                                    rhs = xpb3[:, lr : lr + ROWS, kw : kw + W]
                                nc.tensor.matmul(
                                    ps[:], lhsT, rhs,
                                    start=(k == 0), stop=(k == KS * KS - 1),
                                )
                        # psum -> sbuf with per-channel bias add
                        if last_img and half == 1 and rc == RC - 1:
                            # final chunk: two 4-row pieces so the very last
                            # copy+DMA latency is halved
                            HNF = NF // 2
                            for piece in range(2):
                                lo = half * PIX + rc * NF + piece * HNF
                                nc.vector.tensor_scalar_add(
                                    out_sb[:, lo : lo + HNF],
                                    ps[:, piece * HNF : (piece + 1) * HNF],
                                    bias_sb[:, half : half + 1],
                                )
                                r0 = rc * ROWS + piece * (ROWS // 2)
                                nc.sync.dma_start(
                                    y_d[n, 128:256, r0 : r0 + ROWS // 2, :]
                                    .rearrange("c h w -> c (h w)"),
                                    out_sb[:, lo : lo + HNF],
                                )
                            continue
                        nc.vector.tensor_scalar_add(
                            out_sb[:, half * PIX + rc * NF : half * PIX + (rc + 1) * NF],
                            ps[:],
                            bias_sb[:, half : half + 1],
                        )
                        if last_img and half == 1:
                            # fine-grained tail on the (now idle) sync ring
                            nc.sync.dma_start(
                                y_d[n, 128:256, rc * ROWS : (rc + 1) * ROWS, :]
                                .rearrange("c h w -> c (h w)"),
                                out_sb[:, half * PIX + rc * NF : half * PIX + (rc + 1) * NF],
                            )
                    if not (last_img and half == 1):
                        eng = nc.scalar if half == 0 else nc.gpsimd
                        eng.dma_start(
                            y_d[n, half * 128 : (half + 1) * 128].rearrange("c h w -> c (h w)"),
                            out_sb[:, half * PIX : (half + 1) * PIX],
                        )
    nc.compile()
    return nc


def _get_nc():
    if "nc" not in _CACHE:
        _CACHE["nc"] = _build()
    return _CACHE["nc"]


def _prep_inputs(x, weight, bias):
    # fp16 on host: halves input DMA bytes and drops the on-device casts;
    # same rounding the device cast would apply
    x = np.ascontiguousarray(np.asarray(x, dtype=np.float32).astype(np.float16))
    # [co, ci, kh, kw] -> [ci, half, kh*kw, co_half], half-major so the half-0
    # block is contiguous and can be DMA'd first
    w_t = np.ascontiguousarray(
        np.transpose(np.asarray(weight, dtype=np.float32), (1, 2, 3, 0))
        .reshape(C_IN, KS * KS, 2, 128)
        .transpose(0, 2, 1, 3)
        .astype(np.float16)
    )
    b = np.ascontiguousarray(bias, dtype=np.float32)
    return x, w_t, b


def kernel(x, weight, bias):
    from concourse.bass_utils import run_bass_kernel_spmd

    x, w_t, b = _prep_inputs(x, weight, bias)
    nc = _get_nc()
    in_maps = [
        {"x": x[i * N_PER : (i + 1) * N_PER], "w": w_t, "b": b}
        for i in range(N_CORES)
    ]
    res = run_bass_kernel_spmd(nc, in_maps, list(range(N_CORES)))
    y = np.concatenate([res.results[i]["y"] for i in range(N_CORES)], axis=0)
    return y



# revision 11
# speedup vs baseline: 1.0006x; 1.0006x over previous
"""Conv2d(128->256, 3x3, pad 1, stride 1) on 32x56x56 fp32, for 8 trn2 cores.

Strategy: data-parallel over batch N=32 -> 4 images/core. Per core a
Winograd F(2,3)-along-H implicit GEMM: output rows are produced in pairs
(2t, 2t+1) from 4 row-combinations of the input (v0..v3); each (v_a, kw)
pair is one [128ci x 128co] matmul tap, so a row-pair costs 12 taps of
128-contraction instead of direct conv's 18 -> 2/3 the tensor cycles.

Per chunk of 7 row-pairs (free dim 392 <= 512 PSUM bank) the 12 taps
accumulate into four PSUM tiles m0..m3 (kw taps accumulate, a-taps are
separate banks; 8 banks = double buffer). The inverse transform
  y_even = (m0 + bias) + m1 + m2   (Vector engine, scalar_tensor_tensor+tt)
  y_odd  = (m1 + bias) - m2 - m3   (Pool engine,   scalar_tensor_tensor+tt)
runs split across the two idle ALU engines so it hides under the matmul
stream. The row transform v is computed on Vector in fp16 directly from
the raw (unpadded) image with strided APs; edge pairs t=0/t=27 get small
fix-up ops and the left/right zero pad columns are memset once.

Matmuls run in fp16 (inputs ~N(0,1): ~3e-4 rel err) with fp32 PSUM.
Weights are host-transformed (G g per kh, laid out half-major) so the
half-0 weight DMA lands first; x input rides the SP ring, weights/bias
and full-image outputs the ACT ring (Pool's SWDGE would steal Q7 cycles
from the y_odd transform). The v ops for image n+1 are emitted between
image n's half-0 and half-1 chunks so the PE never waits on Vector at
image boundaries; the last image's half-1 is drained per-chunk on the
idle SP ring with the final chunk split in two.
"""
import numpy as np
from contextlib import ExitStack

N_FULL, C_IN, H, W = 32, 128, 56, 56
C_OUT, KS = 256, 3
N_CORES = 8
N_PER = N_FULL // N_CORES          # 4 images per core
PIX = H * W                         # 3136
NT = H // 2                         # 28 output row-pairs
TP = 7                              # row-pairs per psum chunk
NCH = NT // TP                      # 4 chunks per (image, half)
NF = TP * W                         # 392 free elems per matmul
NTAP = 12                           # 4 winograd row-taps x 3 kw

_CACHE = {}


def _build():
    import concourse.tile as tile
    from concourse import mybir, bacc

    f32 = mybir.dt.float32
    f16 = mybir.dt.float16
    ADD = mybir.AluOpType.add
    SUB = mybir.AluOpType.subtract

    nc = bacc.Bacc("TRN2", target_bir_lowering=False, debug=False)
    x_d = nc.dram_tensor("x", [N_PER, C_IN, H, W], f16, kind="ExternalInput").ap()
    # host-pretransformed winograd weights: [ci, half, tap=a*3+kw, co_half]
    w_d = nc.dram_tensor("w", [C_IN, 2, NTAP, 128], f16, kind="ExternalInput").ap()
    b_d = nc.dram_tensor("b", [C_OUT], f32, kind="ExternalInput").ap()
    y_d = nc.dram_tensor("y", [N_PER, C_OUT, H, W], f32, kind="ExternalOutput").ap()

    with tile.TileContext(nc) as tc:
        with ExitStack() as ctx:
            wp = ctx.enter_context(tc.tile_pool(name="wp", bufs=1))
            xr = ctx.enter_context(tc.tile_pool(name="xr", bufs=2))
            vp = ctx.enter_context(tc.tile_pool(name="vp", bufs=2))
            st0 = ctx.enter_context(tc.tile_pool(name="st0", bufs=4))
            st1 = ctx.enter_context(tc.tile_pool(name="st1", bufs=4))
            sa0 = ctx.enter_context(tc.tile_pool(name="sa0", bufs=4))
            sa1 = ctx.enter_context(tc.tile_pool(name="sa1", bufs=4))
            # PSUM as 2-bank pair tiles: m0|m1 and m2|m3 (each matmul target
            # stays inside one bank; paired reads use a stride-512 AP)
            pp = ctx.enter_context(tc.tile_pool(name="pp", bufs=2, space="PSUM"))
            op = ctx.enter_context(tc.tile_pool(name="op", bufs=2))

            # Weight half 0 first on the ACT ring: it gates the first matmul.
            w_r = wp.tile([C_IN, 2 * NTAP * 128], f16)
            w_r4 = w_r[:].rearrange("p (h k co) -> p h k co", h=2, k=NTAP)
            nc.scalar.dma_start(
                w_r4[:, 0], w_d[:, 0].rearrange("ci k co -> ci (k co)")
            )

            # PE warmup: dummy matmuls while the head DMAs land, so the HAM
            # clock gate opens before the first real matmul issues.
            wu = wp.tile([128, NF], f16)
            nc.vector.memset(wu[:], 0.0)
            wups = pp.tile([128, 1024], f32, name="pm01")
            for _ in range(9):
                nc.tensor.matmul(
                    wups[:, 0:NF], wu[:, 0:128], wu[:], start=True, stop=True
                )

            bias_sb = wp.tile([128, 2], f32)

            x_tiles = [None] * N_PER
            v_tiles = [None] * N_PER
            o_tiles = [None] * N_PER

            def emit_x(n):
                # raw image in one tile, two row-slices so the v ops for the
                # top half unblock early
                xt = xr.tile([C_IN, PIX], f16)
                x3 = xt[:].rearrange("p (h w) -> p h w", w=W)
                nc.sync.dma_start(
                    x3[:, 0:29, :], x_d[n, :, 0:29, :].rearrange("c h w -> c h w")
                )
                nc.sync.dma_start(
                    x3[:, 29:56, :], x_d[n, :, 29:56, :].rearrange("c h w -> c h w")
                )
                x_tiles[n] = xt

            def emit_v(n):
                # v[a, t, 0:58]: winograd row transform of padded rows
                # 2t..2t+3; cols 0/57 are the zero pad, cols 1..56 from raw x.
                xt = x_tiles[n]
                vt = vp.tile([C_IN, 4 * NT * 58], f16)
                v4 = vt[:].rearrange("p (a t w) -> p a t w", a=4, t=NT)
                v3 = vt[:].rearrange("p (at w) -> p at w", w=58)
                nc.vector.memset(v3[:, :, 0:1], 0.0)
                nc.vector.memset(v3[:, :, 57:58], 0.0)
                x3 = xt[:].rearrange("p (h w) -> p h w", w=W)
                x4 = xt[:].rearrange("p (t r w) -> p t r w", r=2, w=W)
                for T0, TN in ((0, 14), (14, 14)):
                    te = T0 + TN
                    ev = x4[:, T0:te, 0, :]   # rows 2t
                    od = x4[:, T0:te, 1, :]   # rows 2t+1
                    # v1 = x[2t] + x[2t+1];  v2 = x[2t+1] - x[2t]
                    nc.vector.tensor_tensor(v4[:, 1, T0:te, 1:57], ev, od, ADD)
                    nc.vector.tensor_tensor(v4[:, 2, T0:te, 1:57], od, ev, SUB)
                    # v0 = x[2t-1] - x[2t+1]   (t=0: row -1 is the zero pad)
                    t0, tn = (1, TN - 1) if T0 == 0 else (T0, TN)
                    if T0 == 0:
                        nc.vector.tensor_scalar_mul(
                            v4[:, 0, 0:1, 1:57], x3[:, 1:2, :], -1.0
                        )
                    sl = xt[:, (2 * t0 - 1) * W : (2 * t0 - 1) * W + tn * 2 * W]
                    sl = sl.rearrange("p (t q) -> p t q", q=2 * W)
                    nc.vector.tensor_tensor(
                        v4[:, 0, t0:t0 + tn, 1:57], sl[:, :, 0:W],
                        x4[:, t0:t0 + tn, 1, :], SUB,
                    )
                    # v3 = x[2t] - x[2t+2]    (t=27: row 56 is the zero pad)
                    t3, n3 = (T0, TN) if T0 == 0 else (T0, TN - 1)
                    nc.vector.tensor_tensor(
                        v4[:, 3, t3:t3 + n3, 1:57], x4[:, t3:t3 + n3, 0, :],
                        x4[:, t3 + 1:t3 + 1 + n3, 0, :], SUB,
                    )
                    if T0 != 0:
                        nc.vector.tensor_copy(v4[:, 3, 27:28, 1:57], x3[:, 54:55, :])
                v_tiles[n] = vt

            def emit_chunk(n, half, c, fine_dma, split):
                v4 = v_tiles[n][:].rearrange("p (a t w) -> p a t w", a=4, t=NT)
                o4 = o_tiles[n][:].rearrange(
                    "p (x r w) -> p x r w", r=2, w=W
                )  # x = half*28 + t
                pm01 = pp.tile([128, 1024], f32, name="pm01")
                pm23 = pp.tile([128, 1024], f32, name="pm23")
                m = []
                for a in range(4):
                    pt = (pm01, pm23)[a // 2]
                    lo = (a % 2) * 512
                    for kw in range(KS):
                        nc.tensor.matmul(
                            pt[:, lo : lo + NF],
                            w_r4[:, half, a * KS + kw, :],
                            v4[:, a, TP * c : TP * c + TP, kw : kw + W],
                            start=(kw == 0), stop=(kw == KS - 1),
                        )
                    m.append(
                        pt[:, lo : lo + NF].rearrange("p (t w) -> p t w", w=W)
                    )
                base = half * NT + TP * c
                bsc = bias_sb[:, half : half + 1]
                pieces = ((0, 3), (3, 4)) if split else ((0, TP),)
                for p0, pn in pieces:
                    # PSUM access rules: ALU ops read at most one PSUM input
                    # and gpsimd none at all. ACT (idle otherwise) biases
                    # m0|m1 and evacuates m2|m3 with paired stride-512 APs;
                    # DVE finishes y_even against PSUM, Pool y_odd in SBUF.
                    pr01 = pm01[:].rearrange("p (k x) -> p k x", k=2)
                    pr23 = pm23[:].rearrange("p (k x) -> p k x", k=2)
                    sel = slice(p0 * W, (p0 + pn) * W)
                    a01 = sa0.tile([128, 2 * pn * W], f32)
                    a01v = a01[:].rearrange("p (k x) -> p k x", k=2)
                    nc.scalar.add(a01v, pr01[:, :, sel], bsc)
                    c23 = sa1.tile([128, 2 * pn * W], f32)
                    c23v = c23[:].rearrange("p (k x) -> p k x", k=2)
                    nc.scalar.copy(c23v, pr23[:, :, sel])
                    s0 = st0.tile([128, pn * W], f32)
                    s03 = s0[:].rearrange("p (t w) -> p t w", w=W)
                    a0v = a01v[:, 0, :].rearrange("p (t w) -> p t w", w=W)
                    a1v = a01v[:, 1, :].rearrange("p (t w) -> p t w", w=W)
                    c2v = c23v[:, 0, :].rearrange("p (t w) -> p t w", w=W)
                    c3v = c23v[:, 1, :].rearrange("p (t w) -> p t w", w=W)
                    nc.vector.tensor_tensor(s03, a0v, m[1][:, p0:p0 + pn], ADD)
                    nc.vector.tensor_tensor(
                        o4[:, base + p0 : base + p0 + pn, 0, :], s03,
                        m[2][:, p0:p0 + pn], ADD,
                    )
                    s1 = st1.tile([128, pn * W], f32)
                    s13 = s1[:].rearrange("p (t w) -> p t w", w=W)
                    nc.gpsimd.tensor_tensor(s13, a1v, c2v, SUB)
                    nc.gpsimd.tensor_tensor(
                        o4[:, base + p0 : base + p0 + pn, 1, :], s13, c3v, SUB,
                    )
                    if fine_dma:
                        r0 = 2 * (TP * c + p0)
                        nc.sync.dma_start(
                            y_d[n, half * 128 : (half + 1) * 128, r0 : r0 + 2 * pn, :]
                            .rearrange("c h w -> c (h w)"),
                            o_tiles[n][:, (half * H + r0) * W : (half * H + r0 + 2 * pn) * W],
                        )

            emit_x(0)
            # weight half 1 + bias after image-0's input DMAs are queued
            nc.scalar.dma_start(
                w_r4[:, 1], w_d[:, 1].rearrange("ci k co -> ci (k co)")
            )
            nc.scalar.dma_start(bias_sb[:], b_d.rearrange("(h p) -> p h", h=2))
            emit_v(0)

            for n in range(N_PER):
                o_tiles[n] = op.tile([128, 2 * PIX], f32, name="osb")
                for half in range(2):
                    if half == 1 and n + 1 < N_PER:
                        emit_x(n + 1)
                        emit_v(n + 1)
                    last = n == N_PER - 1 and half == 1
                    for c in range(NCH):
                        emit_chunk(n, half, c, fine_dma=last, split=last and c == NCH - 1)
                    if not last:
                        nc.scalar.dma_start(
                            y_d[n, half * 128 : (half + 1) * 128]
                            .rearrange("c h w -> c (h w)"),
                            o_tiles[n][:, half * PIX : (half + 1) * PIX],
                        )
    nc.compile()
    return nc


def _get_nc():
    if "nc" not in _CACHE:
        _CACHE["nc"] = _build()
    return _CACHE["nc"]


def _prep_inputs(x, weight, bias):
    # fp16 on host: halves input DMA bytes and drops the on-device casts
    x = np.ascontiguousarray(np.asarray(x, dtype=np.float32).astype(np.float16))
    # winograd F(2,3) weight transform along kh: u = G g, laid out
    # [ci, half, tap=a*3+kw, co_half] half-major so half 0 can be DMA'd first
    w = np.asarray(weight, dtype=np.float32)  # [co, ci, kh, kw]
    g0, g1, g2 = w[:, :, 0, :], w[:, :, 1, :], w[:, :, 2, :]
    u = np.stack([g0, (g0 + g1 + g2) * 0.5, (g0 - g1 + g2) * 0.5, g2])  # [a,co,ci,kw]
    u = u.transpose(2, 0, 3, 1).reshape(C_IN, NTAP, C_OUT)  # [ci, a*3+kw, co]
    w_t = np.ascontiguousarray(
        u.reshape(C_IN, NTAP, 2, 128).transpose(0, 2, 1, 3).astype(np.float16)
    )
    b = np.ascontiguousarray(bias, dtype=np.float32)
    return x, w_t, b


def kernel(x, weight, bias):
    from concourse.bass_utils import run_bass_kernel_spmd

    x, w_t, b = _prep_inputs(x, weight, bias)
    nc = _get_nc()
    in_maps = [
        {"x": x[i * N_PER : (i + 1) * N_PER], "w": w_t, "b": b}
        for i in range(N_CORES)
    ]
    res = run_bass_kernel_spmd(nc, in_maps, list(range(N_CORES)))
    y = np.concatenate([res.results[i]["y"] for i in range(N_CORES)], axis=0)
    return y
